# revision 1
# baseline (speedup 1.0000x reference)
"""Trainium2 Bass kernel for DeepFunnelTransactionMLP.

MLP funnel 15->30->60->90->120->90->60->30->15->10->5->1 (ReLU between,
sigmoid at the end) over a batch of 524288 rows, fp32.

Strategy
--------
Pure data parallel: 8 cores x 65536 rows. On each core, activations are
kept feature-major (features on SBUF partitions, batch streaming on the
free dim), so every layer is one (or two) matmul(s) with the weight as
the stationary operand. Small layers are packed block-diagonally: e.g.
layer1 (15->30) processes 4 independent batch chunks in a single matmul
(4x15 input rows -> 4x30 output rows). Bias+ReLU are fused into single
ScalarE activation / VectorE tensor_scalar instructions reading PSUM.

Host side does the free work: transposing/packing x, building the
block-diagonal weights, and unpermuting the output.
"""

import os
import sys

# The bass PJRT path needs the axon jax platform; undo a cpu-only pin if one
# is set (harmless when jax was already imported by the caller).
if os.environ.get("JAX_PLATFORMS") not in (None, "", "axon", "axon,cpu"):
    os.environ["JAX_PLATFORMS"] = ""

sys.path.insert(0, "/opt/trn_rl_repo")

import numpy as np

import concourse.bacc as bacc
import concourse.mybir as mybir
from concourse.bass_utils import run_bass_kernel_spmd
from concourse.tile import TileContext

_DIMS = [15, 30, 60, 90, 120, 90, 60, 30, 15, 10, 5, 1]
NCORES = 8
B = 524288
BC = B // NCORES  # 65536 rows per core
S = 4096  # super-tile rows
NST = BC // S  # 16 super-tiles per core
F32 = mybir.dt.float32
F32R = mybir.dt.float32r

# Weight variants: (layer l (1-based), K, M, [(koff, moff), ...]).
# lhsT[koff+k, moff+m] = W_l[m, k] for each block; matmul out = lhsT.T @ rhs.
_VARIANTS = [
    ("w1", 1, 60, 120, [(15 * j, 30 * j) for j in range(4)]),
    ("w2A", 2, 60, 120, [(0, 0), (30, 60)]),
    ("w2B", 2, 120, 120, [(60, 0), (90, 60)]),
    ("w3A", 3, 60, 90, [(0, 0)]),
    ("w3B", 3, 120, 90, [(60, 0)]),
    ("w4", 4, 90, 120, [(0, 0)]),
    ("w5", 5, 120, 90, [(0, 0)]),
    ("w6A", 6, 90, 60, [(0, 0)]),
    ("w6B", 6, 90, 120, [(0, 60)]),
    ("w7A", 7, 120, 60, [(0, 0), (60, 30)]),
    ("w7B", 7, 120, 120, [(0, 60), (60, 90)]),
    ("w8A", 8, 120, 60, [(30 * j, 15 * j) for j in range(4)]),
    ("w8B", 8, 120, 120, [(30 * j, 60 + 15 * j) for j in range(4)]),
    ("w9", 9, 120, 80, [(15 * j, 10 * j) for j in range(8)]),
    ("w10", 10, 80, 40, [(10 * j, 5 * j) for j in range(8)]),
    ("w11", 11, 40, 8, [(5 * j, j) for j in range(8)]),
]
_VIDX = {name: i for i, (name, *_) in enumerate(_VARIANTS)}
# tight column packing: variant i starts at the cumulative sum of M widths
_WOFF = {}
_wc = 0
for name, _, _, M, _ in _VARIANTS:
    _WOFF[name] = _wc
    _wc += M
W_COLS = _wc

# Bias layouts: (layer l, tile count) -> packed [tile*dim] at column l-1.
_BIAS_TILES = [4, 2, 1, 1, 1, 2, 4, 8, 8, 8, 8]


def _pack_weights(Ws):
    w = np.zeros((128, W_COLS), dtype=np.float32)
    for name, l, K, M, blocks in _VARIANTS:
        Wl = Ws[l - 1]  # [fan_out, fan_in]
        fo, fi = Wl.shape
        c0 = _WOFF[name]
        for koff, moff in blocks:
            w[koff : koff + fi, c0 + moff : c0 + moff + fo] = Wl.T
    return w


def _pack_biases(bs):
    b = np.zeros((128, 16), dtype=np.float32)
    for l, (bl, nt) in enumerate(zip(bs, _BIAS_TILES)):
        v = np.tile(bl, nt)
        b[: v.shape[0], l] = v
    return b


def _out_map():
    """batch-row (within a super-tile) for output element [group j, col n]."""
    M0 = np.arange(S).reshape(4, S // 4)
    M1 = M0
    M2 = np.empty((2, 2048), dtype=np.int64)
    for t in range(2):
        M2[:, 512 * t : 512 * (t + 1)] = M1[0:2, 512 * t : 512 * (t + 1)]
        M2[:, 1024 + 512 * t : 1024 + 512 * (t + 1)] = M1[2:4, 512 * t : 512 * (t + 1)]
    M3 = np.empty((1, 4096), dtype=np.int64)
    for u in range(4):
        M3[0, 512 * u : 512 * (u + 1)] = M2[0, 512 * u : 512 * (u + 1)]
        M3[0, 2048 + 512 * u : 2048 + 512 * (u + 1)] = M2[1, 512 * u : 512 * (u + 1)]
    M5 = M3
    M6 = np.empty((2, 2048), dtype=np.int64)
    for w in range(4):
        M6[0, 512 * w : 512 * (w + 1)] = M5[0, 1024 * w : 1024 * w + 512]
        M6[1, 512 * w : 512 * (w + 1)] = M5[0, 1024 * w + 512 : 1024 * w + 1024]
    M7 = np.empty((4, 1024), dtype=np.int64)
    for w in range(2):
        M7[0:2, 512 * w : 512 * (w + 1)] = M6[0:2, 1024 * w : 1024 * w + 512]
        M7[2:4, 512 * w : 512 * (w + 1)] = M6[0:2, 1024 * w + 512 : 1024 * w + 1024]
    M8 = np.empty((8, 512), dtype=np.int64)
    M8[0:4, :] = M7[0:4, 0:512]
    M8[4:8, :] = M7[0:4, 512:1024]
    return M8


_NC_CACHE = None


def _build_nc():
    global _NC_CACHE
    if _NC_CACHE is not None:
        return _NC_CACHE

    nc = bacc.Bacc("TRN2", target_bir_lowering=False, debug=False, num_devices=NCORES)
    xt = nc.dram_tensor("xt", [60, BC // 4], F32R, kind="ExternalInput")
    wd = nc.dram_tensor("w", [128, W_COLS], F32R, kind="ExternalInput")
    bd = nc.dram_tensor("b", [128, 16], F32, kind="ExternalInput")
    y = nc.dram_tensor("y", [8, BC // 8], F32, kind="ExternalOutput")

    with TileContext(nc) as tc:
        with (
            tc.tile_pool(name="const", bufs=1) as cpool,
            tc.tile_pool(name="act", bufs=1) as apool,
            tc.tile_pool(name="act2", bufs=2) as apool2,
            tc.tile_pool(name="io", bufs=3) as iopool,
            tc.tile_pool(name="psum", bufs=4, space="PSUM") as pspool,
        ):
            wsb = cpool.tile([128, W_COLS], F32R, tag="w")
            bsb = cpool.tile([128, 16], F32, tag="b")
            nc.sync.dma_start(out=wsb[:], in_=wd[:])
            nc.sync.dma_start(out=bsb[:], in_=bd[:])

            # Dummy sigmoid first: loads the sigmoid_and_others table set
            # (which also serves Relu) once during startup, instead of a
            # ~1.3us mid-pipeline table switch at the first real sigmoid.
            scr = cpool.tile([1, 1], F32, tag="scr", name="scr")
            nc.vector.memset(scr[:], 0.0)
            nc.scalar.activation(scr[:], scr[:],
                                 mybir.ActivationFunctionType.Sigmoid,
                                 bias=0.0, scale=1.0)

            def w_ap(name):
                _, _, K, M, _ = _VARIANTS[_VIDX[name]]
                c0 = _WOFF[name]
                return wsb[0:K, c0 : c0 + M]

            def b_ap(l, P):
                return bsb[0:P, l - 1 : l]

            eng_busy = [5500.0, 0.0]  # modeled ns on [ScalarE, VectorE]; ScalarE pre-charged to offset forced sigmoid work

            def round_(mms, h_slice, l, P, cols, force_scalar=False):
                """One psum tile: 512-col matmuls (slice advances on start=True),
                then a fused bias+relu drain on the less-loaded act engine."""
                ps = pspool.tile([128, 1024], F32, tag="ps")
                q = -1
                for wname, rhs, start, stop in mms:
                    _, _, K, M, _ = _VARIANTS[_VIDX[wname]]
                    if start:
                        q += 1
                    nc.tensor.matmul(ps[0:M, 512 * q : 512 * q + 512], w_ap(wname),
                                     rhs, start=start, stop=stop)
                if force_scalar or eng_busy[0] + cols / 1.2 + 143 <= eng_busy[1] + cols / 0.96 + 125:
                    nc.scalar.activation(
                        h_slice, ps[0:P, 0:cols], mybir.ActivationFunctionType.Relu,
                        bias=b_ap(l, P), scale=1.0,
                    )
                    eng_busy[0] += cols / 1.2 + 143
                else:
                    nc.vector.tensor_scalar(
                        out=h_slice, in0=ps[0:P, 0:cols],
                        scalar1=b_ap(l, P), scalar2=0.0,
                        op0=mybir.AluOpType.add, op1=mybir.AluOpType.max,
                    )
                    eng_busy[1] += cols / 0.96 + 125

            def build_t1(st, d):
                """L1-L2: 3 rounds."""
                rounds = []

                def r_dma():
                    c0 = (S // 4) * st
                    d["h0"] = iopool.tile([60, 1024], F32R, tag="h0", name="h0")
                    nc.sync.dma_start(out=d["h0"][:], in_=xt[:, c0 : c0 + 1024])
                    d["h1"] = apool2.tile([120, 1024], F32R, tag="h1", name="h1")
                    round_([("w1", d["h0"][0:60, 0:512], True, True),
                            ("w1", d["h0"][0:60, 512:1024], True, True)],
                           d["h1"][:, :], 1, 120, 1024)
                rounds.append(r_dma)

                def r_l2(half):
                    def f():
                        if half == 0:
                            d["h2"] = apool2.tile([120, 2048], F32R, tag="h2", name="h2")
                            round_([("w2A", d["h1"][0:60, 0:512], True, True),
                                    ("w2A", d["h1"][0:60, 512:1024], True, True)],
                                   d["h2"][:, 0:1024], 2, 120, 1024)
                        else:
                            round_([("w2B", d["h1"][0:120, 0:512], True, True),
                                    ("w2B", d["h1"][0:120, 512:1024], True, True)],
                                   d["h2"][:, 1024:2048], 2, 120, 1024)
                    return f
                rounds += [r_l2(0), r_l2(1)]

                return rounds

            def build_t1b(st, d):
                """L3: 4 rounds."""
                rounds = []

                def r_l3(r):
                    def f():
                        if r == 0:
                            d["h3"] = apool2.tile([90, 4096], F32R, tag="h3", name="h3")
                        if r < 2:
                            round_([("w3A", d["h2"][0:60, 1024 * r : 1024 * r + 512], True, True),
                                    ("w3A", d["h2"][0:60, 1024 * r + 512 : 1024 * (r + 1)], True, True)],
                                   d["h3"][:, 1024 * r : 1024 * (r + 1)], 3, 90, 1024)
                        else:
                            rr = r - 2
                            round_([("w3B", d["h2"][0:120, 1024 * rr : 1024 * rr + 512], True, True),
                                    ("w3B", d["h2"][0:120, 1024 * rr + 512 : 1024 * (rr + 1)], True, True)],
                                   d["h3"][:, 2048 + 1024 * rr : 2048 + 1024 * (rr + 1)], 3, 90, 1024)
                    return f
                rounds += [r_l3(r) for r in range(4)]
                return rounds

            def build_t2(st, d):
                """L4: 4 rounds."""
                rounds = []

                def r_l4(r):
                    def f():
                        if r == 0:
                            d["h4"] = apool.tile([120, 4096], F32R, tag="h4", name="h4")
                        round_([("w4", d["h3"][0:90, 1024 * r : 1024 * r + 512], True, True),
                                ("w4", d["h3"][0:90, 1024 * r + 512 : 1024 * (r + 1)], True, True)],
                               d["h4"][:, 1024 * r : 1024 * (r + 1)], 4, 120, 1024)
                    return f
                rounds += [r_l4(r) for r in range(4)]

                return rounds

            def build_t2b(st, d):
                """L5: 4 rounds."""
                rounds = []

                def r_l5(r):
                    def f():
                        if r == 0:
                            d["h5"] = apool2.tile([90, 4096], F32R, tag="h5", name="h5")
                        round_([("w5", d["h4"][0:120, 1024 * r : 1024 * r + 512], True, True),
                                ("w5", d["h4"][0:120, 1024 * r + 512 : 1024 * (r + 1)], True, True)],
                               d["h5"][:, 1024 * r : 1024 * (r + 1)], 5, 90, 1024)
                    return f
                rounds += [r_l5(r) for r in range(4)]
                return rounds

            def build_t3(st, d):
                """L6-L11: 7 rounds."""
                rounds = []

                def r_l6(r):
                    def f():
                        if r == 0:
                            d["h6"] = apool2.tile([120, 2048], F32R, tag="h6", name="h6")
                        mms = []
                        for q in range(2):
                            w = 2 * r + q
                            mms.append(("w6A", d["h5"][0:90, 1024 * w : 1024 * w + 512], True, False))
                            mms.append(("w6B", d["h5"][0:90, 1024 * w + 512 : 1024 * (w + 1)], False, True))
                        round_(mms, d["h6"][:, 1024 * r : 1024 * (r + 1)], 6, 120, 1024)
                    return f
                rounds += [r_l6(r) for r in range(2)]

                def r_l7():
                    d["h7"] = apool2.tile([120, 1024], F32R, tag="h7", name="h7")
                    mms = []
                    for w in range(2):
                        mms.append(("w7A", d["h6"][0:120, 1024 * w : 1024 * w + 512], True, False))
                        mms.append(("w7B", d["h6"][0:120, 1024 * w + 512 : 1024 * (w + 1)], False, True))
                    round_(mms, d["h7"][:, :], 7, 120, 1024)
                rounds.append(r_l7)

                def r_l8():
                    d["h8"] = apool2.tile([120, 512], F32R, tag="h8", name="h8")
                    round_([("w8A", d["h7"][0:120, 0:512], True, False),
                            ("w8B", d["h7"][0:120, 512:1024], False, True)],
                           d["h8"][:, :], 8, 120, 512)
                rounds.append(r_l8)

                def r_l9():
                    d["h9"] = apool2.tile([80, 512], F32R, tag="h9", name="h9")
                    round_([("w9", d["h8"][0:120, :], True, True)], d["h9"][:, :], 9, 80, 512)
                rounds.append(r_l9)

                def r_l10():
                    d["h10"] = apool2.tile([40, 512], F32R, tag="h10", name="h10")
                    round_([("w10", d["h9"][0:80, :], True, True)], d["h10"][:, :], 10, 40, 512)
                rounds.append(r_l10)

                def r_l11():
                    ps = pspool.tile([128, 1024], F32, tag="ps", name="ps")
                    nc.tensor.matmul(ps[0:8, 0:512], w_ap("w11"), d["h10"][0:40, :],
                                     start=True, stop=True)
                    osb = iopool.tile([8, 512], F32, tag="osb", name="osb")
                    nc.scalar.activation(
                        osb[:, :], ps[0:8, 0:512], mybir.ActivationFunctionType.Sigmoid,
                        bias=b_ap(11, 8), scale=1.0,
                    )
                    eng_busy[0] += 512 / 1.2 + 143
                    nc.sync.dma_start(out=y[:, 512 * st : 512 * (st + 1)], in_=osb[:, :])
                rounds.append(r_l11)
                return rounds

            # 3-way software pipeline: epoch e runs (L1-L3)(e), (L4-L5)(e-1),
            # (L6-L11)(e-2) round-robin, so three super-tiles' serial layer
            # chains overlap in every engine's in-order queue.
            from itertools import zip_longest

            dicts = [dict() for _ in range(NST)]
            for e in range(NST + 2):
                parts = []
                if e < NST:
                    parts.append(build_t1(e, dicts[e]) + build_t1b(e, dicts[e]))
                if 0 <= e - 1 < NST:
                    parts.append(build_t2(e - 1, dicts[e - 1]) + build_t2b(e - 1, dicts[e - 1]))
                if 0 <= e - 2 < NST:
                    parts.append(build_t3(e - 2, dicts[e - 2]))
                for grp in zip_longest(*parts):
                    for r in grp:
                        if r is not None:
                            r()

    nc.compile()
    _NC_CACHE = nc
    return nc


def _make_in_maps(inputs):
    x = np.asarray(inputs["x"], dtype=np.float32)
    Ws = [np.asarray(inputs[f"W{i}"], dtype=np.float32) for i in range(1, 12)]
    bs = [np.asarray(inputs[f"b{i}"], dtype=np.float32) for i in range(1, 12)]

    w_pack = _pack_weights(Ws)
    b_pack = _pack_biases(bs)

    in_maps = []
    for c in range(NCORES):
        xc = x[c * BC : (c + 1) * BC]
        # xt[15j+f, (S//4)*st + m] = xc[st*S + j*(S//4) + m, f]
        xt = np.ascontiguousarray(
            xc.reshape(NST, 4, S // 4, _DIMS[0]).transpose(1, 3, 0, 2).reshape(60, BC // 4)
        )
        in_maps.append({"xt": xt, "w": w_pack, "b": b_pack})
    return in_maps


def kernel(**inputs):
    in_maps = _make_in_maps(inputs)
    nc = _build_nc()
    res = run_bass_kernel_spmd(nc, in_maps, list(range(NCORES)))

    omap = _out_map()  # [8, 512] batch row within super-tile
    out = np.empty((B, 1), dtype=np.float32)
    for c in range(NCORES):
        yc = res.results[c]["y"]  # [8, BC//8]
        for st in range(NST):
            blk = np.empty(S, dtype=np.float32)
            blk[omap.ravel()] = yc[:, 512 * st : 512 * (st + 1)].ravel()
            out[c * BC + st * S : c * BC + (st + 1) * S, 0] = blk
    return out



# revision 12
# speedup vs baseline: 1.0280x; 1.0280x over previous
"""Trainium2 Bass kernel for DeepFunnelTransactionMLP.

MLP funnel 15->30->60->90->120->90->60->30->15->10->5->1 (ReLU between,
sigmoid at the end) over a batch of 524288 rows, fp32.

Strategy
--------
Pure data parallel: 8 cores x 65536 rows. On each core, activations are
kept feature-major (features on SBUF partitions, batch streaming on the
free dim), so every layer is one (or two) matmul(s) with the weight as
the stationary operand. Small layers are packed block-diagonally: e.g.
layer1 (15->30) processes 4 independent batch chunks in a single matmul
(4x15 input rows -> 4x30 output rows). Bias+ReLU are fused into single
ScalarE activation / VectorE tensor_scalar instructions reading PSUM.

Host side does the free work: transposing/packing x, building the
block-diagonal weights, and unpermuting the output.
"""

import os
import sys

# The bass PJRT path needs the axon jax platform; undo a cpu-only pin if one
# is set (harmless when jax was already imported by the caller).
if os.environ.get("JAX_PLATFORMS") not in (None, "", "axon", "axon,cpu"):
    os.environ["JAX_PLATFORMS"] = ""

sys.path.insert(0, "/opt/trn_rl_repo")

import numpy as np

import concourse.bacc as bacc
import concourse.mybir as mybir
from concourse.bass_utils import run_bass_kernel_spmd
from concourse.tile import TileContext

_DIMS = [15, 30, 60, 90, 120, 90, 60, 30, 15, 10, 5, 1]
NCORES = 8
B = 524288
BC = B // NCORES  # 65536 rows per core
S = 4096  # super-tile rows
NST = BC // S  # 16 super-tiles per core
F32 = mybir.dt.float32
F32R = mybir.dt.float32r

# Weight variants: (layer l (1-based), K, M, [(koff, moff), ...]).
# lhsT[koff+k, moff+m] = W_l[m, k] for each block; matmul out = lhsT.T @ rhs.
_VARIANTS = [
    ("w1", 1, 60, 120, [(15 * j, 30 * j) for j in range(4)]),
    ("w2A", 2, 60, 120, [(0, 0), (30, 60)]),
    ("w2B", 2, 120, 120, [(60, 0), (90, 60)]),
    ("w3A", 3, 60, 90, [(0, 0)]),
    ("w3B", 3, 120, 90, [(60, 0)]),
    ("w4", 4, 90, 120, [(0, 0)]),
    ("w5", 5, 120, 90, [(0, 0)]),
    ("w6A", 6, 90, 60, [(0, 0)]),
    ("w6B", 6, 90, 120, [(0, 60)]),
    ("w7A", 7, 120, 60, [(0, 0), (60, 30)]),
    ("w7B", 7, 120, 120, [(0, 60), (60, 90)]),
    ("w8A", 8, 120, 60, [(30 * j, 15 * j) for j in range(4)]),
    ("w8B", 8, 120, 120, [(30 * j, 60 + 15 * j) for j in range(4)]),
    ("w9", 9, 120, 80, [(15 * j, 10 * j) for j in range(8)]),
    # L10 output 16-packed: out col c' <- in cols {2c', 2c'+1}; variant m
    # handles in col parity m, writing out partition block 40m:40m+40.
    ("w10A", 10, 80, 80, [(10 * j, 5 * j) for j in range(8)]),
    ("w10B", 10, 80, 80, [(10 * j, 40 + 5 * j) for j in range(8)]),
]
# L11 is bf16 (its matmuls have N=64 < 256, where f32r drops to 4 cyc/row):
# out col c'' <- in cols {4c''+m}; variant m maps in row i -> out partition
# 16m+i. Packed in its own [80, 256] bf16 tile, variant m at cols 64m:64m+64.
_W11_VARIANTS = [(m, [(5 * i, 16 * m + i) for i in range(16)]) for m in range(4)]
_VIDX = {name: i for i, (name, *_) in enumerate(_VARIANTS)}
# tight column packing: variant i starts at the cumulative sum of M widths
_WOFF = {}
_wc = 0
for name, _, _, M, _ in _VARIANTS:
    _WOFF[name] = _wc
    _wc += M
W_COLS = _wc

# Bias layouts: (layer l, tile count) -> packed [tile*dim] at column l-1.
_BIAS_TILES = [4, 2, 1, 1, 1, 2, 4, 8, 8, 16, 64]


def _pack_weights(Ws):
    w = np.zeros((128, W_COLS), dtype=np.float32)
    for name, l, K, M, blocks in _VARIANTS:
        Wl = Ws[l - 1]  # [fan_out, fan_in]
        fo, fi = Wl.shape
        c0 = _WOFF[name]
        for koff, moff in blocks:
            w[koff : koff + fi, c0 + moff : c0 + moff + fo] = Wl.T
    return w


def _pack_w11(W11):
    import ml_dtypes

    w = np.zeros((80, 256), dtype=np.float32)
    for m, blocks in _W11_VARIANTS:
        for koff, moff in blocks:
            w[koff : koff + 5, 64 * m + moff] = W11[0, :]
    return w.astype(ml_dtypes.bfloat16)


def _pack_biases(bs):
    b = np.zeros((128, 16), dtype=np.float32)
    for l, (bl, nt) in enumerate(zip(bs, _BIAS_TILES)):
        v = np.tile(bl, nt)
        b[: v.shape[0], l] = v
    return b


def _out_map():
    """batch-row (within a super-tile) for output element [partition p, col c''].

    y[p, 64*st + c''] = row v(512*(i%8) + 2*(4*c''+m) + i//8) with m=p//16,
    i=p%16, where v swaps the middle two 1024-blocks (the h2/h3 column
    permutation).
    """
    p = np.arange(64)[:, None]
    cpp = np.arange(64)[None, :]
    m, i = p // 16, p % 16
    cprime = 4 * cpp + m
    c = 512 * (i % 8) + 2 * cprime + i // 8
    v = c.copy()
    v[(c >= 1024) & (c < 2048)] += 1024
    v[(c >= 2048) & (c < 3072)] -= 1024
    return v


_NC_CACHE = None


def _build_nc():
    global _NC_CACHE
    if _NC_CACHE is not None:
        return _NC_CACHE

    nc = bacc.Bacc("TRN2", target_bir_lowering=False, debug=False, num_devices=NCORES)
    BF16 = mybir.dt.bfloat16
    xt = nc.dram_tensor("xt", [60, BC // 4], F32R, kind="ExternalInput")
    wd = nc.dram_tensor("w", [128, W_COLS], F32R, kind="ExternalInput")
    w11d = nc.dram_tensor("w11", [80, 256], BF16, kind="ExternalInput")
    bd = nc.dram_tensor("b", [128, 16], F32, kind="ExternalInput")
    y = nc.dram_tensor("y", [64, BC // 64], F32, kind="ExternalOutput")

    with TileContext(nc) as tc:
        with (
            tc.tile_pool(name="const", bufs=1) as cpool,
            tc.tile_pool(name="act", bufs=1) as apool,
            tc.tile_pool(name="act2", bufs=2) as apool2,
            tc.tile_pool(name="io", bufs=3) as iopool,
            tc.tile_pool(name="psum", bufs=4, space="PSUM") as pspool,
        ):
            wsb = cpool.tile([128, W_COLS], F32R, tag="w")
            w11sb = cpool.tile([80, 256], mybir.dt.bfloat16, tag="w11")
            bsb = cpool.tile([128, 16], F32, tag="b")
            # w1 + biases first (small) so L1 can start as soon as x tile 0
            # lands; the bulk of the weights follows behind x tile 0 and is
            # needed only ~1.5us later (first L2 round).
            nc.sync.dma_start(out=wsb[:, 0:120], in_=wd[:, 0:120])
            nc.sync.dma_start(out=bsb[:], in_=bd[:])

            # Dummy sigmoid first: loads the sigmoid_and_others table set
            # (which also serves Relu) once during startup, instead of a
            # ~1.3us mid-pipeline table switch at the first real sigmoid.
            scr = cpool.tile([1, 1], F32, tag="scr", name="scr")
            nc.vector.memset(scr[:], 0.0)
            nc.scalar.activation(scr[:], scr[:],
                                 mybir.ActivationFunctionType.Sigmoid,
                                 bias=0.0, scale=1.0)

            def w_ap(name):
                _, _, K, M, _ = _VARIANTS[_VIDX[name]]
                c0 = _WOFF[name]
                return wsb[0:K, c0 : c0 + M]

            def b_ap(l, P):
                return bsb[0:P, l - 1 : l]

            eng_busy = [1400.0, 0.0]  # modeled ns on [ScalarE, VectorE]; ScalarE pre-charged for the act-table load

            def round_(mms, h_slice, l, P, cols, force_scalar=False):
                """One psum tile: a sequence of matmul groups (each group =
                one start=True..stop=True accumulation over the same psum
                columns; column cursor advances by the group's rhs width on
                stop), then a fused bias+relu drain on the less-loaded act
                engine."""
                ps = pspool.tile([128, 1024], F32, tag="ps")
                cur = 0
                base = 0
                for wname, rhs, start, stop in mms:
                    _, _, K, M, _ = _VARIANTS[_VIDX[wname]]
                    n = rhs.shape[-1]
                    if start:
                        base = cur
                    nc.tensor.matmul(ps[0:M, base : base + n], w_ap(wname),
                                     rhs, start=start, stop=stop)
                    if stop:
                        cur = base + n
                if force_scalar or eng_busy[0] + cols / 1.2 + 185 <= eng_busy[1] + cols / 0.96 + 125:
                    nc.scalar.activation(
                        h_slice, ps[0:P, 0:cols], mybir.ActivationFunctionType.Relu,
                        bias=b_ap(l, P), scale=1.0,
                    )
                    eng_busy[0] += cols / 1.2 + 185
                else:
                    nc.vector.tensor_scalar(
                        out=h_slice, in0=ps[0:P, 0:cols],
                        scalar1=b_ap(l, P), scalar2=0.0,
                        op0=mybir.AluOpType.add, op1=mybir.AluOpType.max,
                    )
                    eng_busy[1] += cols / 0.96 + 125

            def build_t1(st, d):
                """L1-L2: 3 rounds."""
                rounds = []

                def r_dma():
                    c0 = (S // 4) * st
                    d["h0"] = iopool.tile([60, 1024], F32R, tag="h0", name="h0")
                    nc.sync.dma_start(out=d["h0"][:], in_=xt[:, c0 : c0 + 1024])
                    if st == 0:
                        # bulk weights ride behind x tile 0
                        nc.sync.dma_start(out=wsb[:, 120:W_COLS], in_=wd[:, 120:W_COLS])
                        nc.sync.dma_start(out=w11sb[:], in_=w11d[:])
                    d["h1"] = apool2.tile([120, 1024], F32R, tag="h1", name="h1")
                    round_([("w1", d["h0"][0:60, 0:512], True, True),
                            ("w1", d["h0"][0:60, 512:1024], True, True)],
                           d["h1"][:, :], 1, 120, 1024)
                rounds.append(r_dma)

                def r_l2(half):
                    def f():
                        if half == 0:
                            d["h2"] = apool2.tile([120, 2048], F32R, tag="h2", name="h2")
                            round_([("w2A", d["h1"][0:60, 0:512], True, True),
                                    ("w2A", d["h1"][0:60, 512:1024], True, True)],
                                   d["h2"][:, 0:1024], 2, 120, 1024)
                        else:
                            round_([("w2B", d["h1"][0:120, 0:512], True, True),
                                    ("w2B", d["h1"][0:120, 512:1024], True, True)],
                                   d["h2"][:, 1024:2048], 2, 120, 1024)
                    return f
                rounds += [r_l2(0), r_l2(1)]

                return rounds

            def build_t1b(st, d):
                """L3: 4 rounds."""
                rounds = []

                def r_l3(r):
                    def f():
                        if r == 0:
                            d["h3"] = apool2.tile([90, 4096], F32R, tag="h3", name="h3")
                        if r < 2:
                            round_([("w3A", d["h2"][0:60, 1024 * r : 1024 * r + 512], True, True),
                                    ("w3A", d["h2"][0:60, 1024 * r + 512 : 1024 * (r + 1)], True, True)],
                                   d["h3"][:, 1024 * r : 1024 * (r + 1)], 3, 90, 1024)
                        else:
                            rr = r - 2
                            round_([("w3B", d["h2"][0:120, 1024 * rr : 1024 * rr + 512], True, True),
                                    ("w3B", d["h2"][0:120, 1024 * rr + 512 : 1024 * (rr + 1)], True, True)],
                                   d["h3"][:, 2048 + 1024 * rr : 2048 + 1024 * (rr + 1)], 3, 90, 1024)
                    return f
                rounds += [r_l3(r) for r in range(4)]
                return rounds

            def build_t2(st, d):
                """L4: 4 rounds."""
                rounds = []

                def r_l4(r):
                    def f():
                        if r == 0:
                            d["h4"] = apool.tile([120, 4096], F32R, tag="h4", name="h4")
                        round_([("w4", d["h3"][0:90, 1024 * r : 1024 * r + 512], True, True),
                                ("w4", d["h3"][0:90, 1024 * r + 512 : 1024 * (r + 1)], True, True)],
                               d["h4"][:, 1024 * r : 1024 * (r + 1)], 4, 120, 1024)
                    return f
                rounds += [r_l4(r) for r in range(4)]

                return rounds

            def build_t2b(st, d):
                """L5: 4 rounds."""
                rounds = []

                def r_l5(r):
                    def f():
                        if r == 0:
                            d["h5"] = apool2.tile([90, 4096], F32R, tag="h5", name="h5")
                        round_([("w5", d["h4"][0:120, 1024 * r : 1024 * r + 512], True, True),
                                ("w5", d["h4"][0:120, 1024 * r + 512 : 1024 * (r + 1)], True, True)],
                               d["h5"][:, 1024 * r : 1024 * (r + 1)], 5, 90, 1024)
                    return f
                rounds += [r_l5(r) for r in range(4)]
                return rounds

            def build_t3(st, d):
                """L6-L11: 7 rounds."""
                rounds = []

                def r_l6(r):
                    def f():
                        if r == 0:
                            d["h6"] = apool2.tile([120, 2048], F32R, tag="h6", name="h6")
                        mms = []
                        for q in range(2):
                            w = 2 * r + q
                            mms.append(("w6A", d["h5"][0:90, 1024 * w : 1024 * w + 512], True, False))
                            mms.append(("w6B", d["h5"][0:90, 1024 * w + 512 : 1024 * (w + 1)], False, True))
                        round_(mms, d["h6"][:, 1024 * r : 1024 * (r + 1)], 6, 120, 1024)
                    return f
                rounds += [r_l6(r) for r in range(2)]

                def r_l7():
                    d["h7"] = apool2.tile([120, 1024], F32R, tag="h7", name="h7")
                    mms = []
                    for w in range(2):
                        mms.append(("w7A", d["h6"][0:120, 1024 * w : 1024 * w + 512], True, False))
                        mms.append(("w7B", d["h6"][0:120, 1024 * w + 512 : 1024 * (w + 1)], False, True))
                    round_(mms, d["h7"][:, :], 7, 120, 1024)
                rounds.append(r_l7)

                def r_l8():
                    d["h8"] = apool2.tile([120, 512], F32R, tag="h8", name="h8")
                    round_([("w8A", d["h7"][0:120, 0:512], True, False),
                            ("w8B", d["h7"][0:120, 512:1024], False, True)],
                           d["h8"][:, :], 8, 120, 512)
                rounds.append(r_l8)

                def r_l9():
                    d["h9"] = apool2.tile([80, 512], F32R, tag="h9", name="h9")
                    round_([("w9", d["h8"][0:120, :], True, True)], d["h9"][:, :], 9, 80, 512)
                rounds.append(r_l9)

                def r_l10():
                    d["h10"] = apool2.tile([80, 256], mybir.dt.bfloat16, tag="h10", name="h10")
                    round_([("w10A", d["h9"][0:80, 0::2], True, False),
                            ("w10B", d["h9"][0:80, 1::2], False, True)],
                           d["h10"][:, :], 10, 80, 256)
                rounds.append(r_l10)

                def r_l11():
                    ps = pspool.tile([128, 1024], F32, tag="ps", name="ps")
                    for m in range(4):
                        nc.tensor.matmul(ps[0:64, 0:64],
                                         w11sb[0:80, 64 * m : 64 * (m + 1)],
                                         d["h10"][0:80, m::4],
                                         start=(m == 0), stop=(m == 3))
                    osb = iopool.tile([64, 64], F32, tag="osb", name="osb")
                    nc.scalar.activation(
                        osb[:, :], ps[0:64, 0:64], mybir.ActivationFunctionType.Sigmoid,
                        bias=b_ap(11, 64), scale=1.0,
                    )
                    eng_busy[0] += 64 / 1.2 + 185
                    nc.sync.dma_start(out=y[:, 64 * st : 64 * (st + 1)], in_=osb[:, :])
                rounds.append(r_l11)
                return rounds

            # 3-way software pipeline: epoch e runs (L1-L3)(e), (L4-L5)(e-1),
            # (L6-L11)(e-2) round-robin, so three super-tiles' serial layer
            # chains overlap in every engine's in-order queue.
            from itertools import zip_longest

            dicts = [dict() for _ in range(NST)]
            for e in range(NST + 2):
                parts = []
                if e < NST:
                    parts.append(build_t1(e, dicts[e]) + build_t1b(e, dicts[e]))
                if 0 <= e - 1 < NST:
                    parts.append(build_t2(e - 1, dicts[e - 1]) + build_t2b(e - 1, dicts[e - 1]))
                if 0 <= e - 2 < NST:
                    parts.append(build_t3(e - 2, dicts[e - 2]))
                for grp in zip_longest(*parts):
                    for r in grp:
                        if r is not None:
                            r()

    nc.compile()
    _NC_CACHE = nc
    return nc


def _make_in_maps(inputs):
    x = np.asarray(inputs["x"], dtype=np.float32)
    Ws = [np.asarray(inputs[f"W{i}"], dtype=np.float32) for i in range(1, 12)]
    bs = [np.asarray(inputs[f"b{i}"], dtype=np.float32) for i in range(1, 12)]

    w_pack = _pack_weights(Ws)
    w11_pack = _pack_w11(Ws[10])
    b_pack = _pack_biases(bs)

    in_maps = []
    for c in range(NCORES):
        xc = x[c * BC : (c + 1) * BC]
        # xt[15j+f, (S//4)*st + m] = xc[st*S + j*(S//4) + m, f]
        xt = np.ascontiguousarray(
            xc.reshape(NST, 4, S // 4, _DIMS[0]).transpose(1, 3, 0, 2).reshape(60, BC // 4)
        )
        in_maps.append({"xt": xt, "w": w_pack, "w11": w11_pack, "b": b_pack})
    return in_maps


def kernel(**inputs):
    in_maps = _make_in_maps(inputs)
    nc = _build_nc()
    res = run_bass_kernel_spmd(nc, in_maps, list(range(NCORES)))

    omap = _out_map()  # [64, 64] batch row within super-tile
    out = np.empty((B, 1), dtype=np.float32)
    for c in range(NCORES):
        yc = res.results[c]["y"]  # [64, BC//64]
        for st in range(NST):
            blk = np.empty(S, dtype=np.float32)
            blk[omap.ravel()] = yc[:, 64 * st : 64 * (st + 1)].ravel()
            out[c * BC + st * S : c * BC + (st + 1) * S, 0] = blk
    return out

